# revision 94
# baseline (speedup 1.0000x reference)
import sys

for _p in ("/opt/trn_rl_repo", "/opt/trn_rl_repo/concourse"):
    if _p not in sys.path:
        sys.path.insert(0, _p)

import numpy as np
import ml_dtypes
import concourse.bass as bass
import concourse.bacc as bacc
import concourse.mybir as mybir
import concourse.tile as tile

P = 128
D = 512
S = 1600
K = 64
NIMG = 4          # images per core
NCORES = 8
NCH = 13          # 12*128 + 64 = 1600
F32 = mybir.dt.float32
F16 = mybir.dt.float16
F8 = mybir.dt.float8e3   # e3m4
U8 = mybir.dt.uint8
AF = mybir.ActivationFunctionType
OP = mybir.AluOpType
AX = mybir.AxisListType

LN_EIGHTH = -2.0794415416798357  # ln(1/8)
AGG_DELAY = 10
NLG = 8              # logits PSUM slices (one bank)
USE_GPSIMD = True    # offload 4th square group to Pool engine
POOL_UPTO = 13       # Pool handles 4th group only for chunks < POOL_UPTO
XT_FMT = "e3"        # 'e3' | 'e4' | 'f16' for the agg GEMM rhs
SS_OFF = 4           # stagger of next image's ss chunks into current main


def build():
    nc = bacc.Bacc("TRN2", target_bir_lowering=False, debug=False,
                   enable_asserts=True, num_devices=NCORES)
    xt_dram_dt = F16 if XT_FMT == "f16" else U8
    xt_mm_dt = {"e3": F8, "e4": mybir.dt.float8e4, "f16": F16}[XT_FMT]
    # host-relaid layouts:
    #  XN [n, p=d%128, g=d//128, s] fp16   (logits lhsT + squares)
    #  XT [n, p=s%128, c=s//128, d] fp8e3-as-u8  (agg GEMM rhs)
    XN_d = nc.dram_tensor("XN", [NIMG, P, 3, S], F16, kind="ExternalInput").ap()
    XN8_d = nc.dram_tensor("XN8", [NIMG, P, S], U8, kind="ExternalInput").ap()
    XT_d = nc.dram_tensor("XT", [NIMG, P, NCH, D], xt_dram_dt,
                          kind="ExternalInput").ap()
    WT_d = nc.dram_tensor("WT", [P, 4, K], F16, kind="ExternalInput").ap()
    CENT_d = nc.dram_tensor("CENT", [K, D], F16, kind="ExternalInput").ap()
    ONES_d = nc.dram_tensor("ONES", [P, 1], F16, kind="ExternalInput").ap()
    OUT_d = nc.dram_tensor("OUT", [NIMG, K, D], F16, kind="ExternalOutput").ap()

    with tile.TileContext(nc) as tc:
        with tc.tile_pool(name="const", bufs=1) as cpool, \
             tc.tile_pool(name="xn", bufs=3) as xnpool, \
             tc.tile_pool(name="xn8", bufs=3) as xn8pool, \
             tc.tile_pool(name="xt", bufs=3) as xtpool, \
             tc.tile_pool(name="xsqd", bufs=5) as xsqdpool, \
             tc.tile_pool(name="xsqp", bufs=5) as xsqppool, \
             tc.tile_pool(name="nrm", bufs=2) as nrmpool, \
             tc.tile_pool(name="expt", bufs=13) as epool, \
             tc.tile_pool(name="e2", bufs=13) as e2pool, \
             tc.tile_pool(name="fin", bufs=2) as finpool, \
             tc.tile_pool(name="ps_lg", bufs=4, space=bass.MemorySpace.PSUM) as ps_lg, \
             tc.tile_pool(name="ps_agg", bufs=2, space=bass.MemorySpace.PSUM) as ps_agg, \
             tc.tile_pool(name="ps_sm", bufs=2, space=bass.MemorySpace.PSUM) as ps_sm:

            # act set 6 = {ln, exp, square, copy, ...}: one table load total
            nc.scalar.add_instruction(mybir.InstLoadActFuncSet(act_func_set_id=6))
            wt = cpool.tile([P, 4, K], F16)
            cent = cpool.tile([K, D], F16)
            ones = cpool.tile([P, 1], F16)
            ln8 = cpool.tile([P, 1], F32)
            nc.vector.memset(ln8[:], LN_EIGHTH)

            xn_t = [None] * NIMG
            xn8_t = [None] * NIMG
            xt_t = [None] * NIMG
            sm_t = [None] * NIMG
            nrm_t = [None] * NIMG

            def load_image(n, fine=False):
                t = xnpool.tile([P, 3, S], F16, name="xn_t")
                t8 = xn8pool.tile([P, S], U8, name="xn8_t")
                pieces = ((0, 400), (400, 800), (800, 1200), (1200, 1600)) \
                    if fine else ((0, 512), (512, 1024), (1024, 1600))
                for i, (a, b) in enumerate(pieces):
                    nc.sync.dma_start(t[:, :, a:b], XN_d[n, :, :, a:b])
                    if fine and n == 0 and i == 0:
                        # squeeze the tiny ones vector in right after the
                        # first x piece; big consts ride later in the queue
                        nc.sync.dma_start(ones[:], ONES_d[:, :])
                    if i == 0:
                        nc.sync.dma_start(t8[:], XN8_d[n, :, :])
                u = xtpool.tile([P, NCH, D], xt_dram_dt, name="xt_t")
                if fine:
                    nc.sync.dma_start(u[:, 0:7, :], XT_d[n, :, 0:7, :])
                    nc.sync.dma_start(u[:, 7:NCH, :], XT_d[n, :, 7:NCH, :])
                else:
                    nc.sync.dma_start(u[:], XT_d[n, :, :, :])
                xn_t[n] = t
                xn8_t[n] = t8
                xt_t[n] = u

            xsq_t = {}

            def ss_sq2(n, jlo, dve_only=False):
                # squares for a PAIR of chunks [jlo, jlo+1] (last chunk solo):
                # DVE does d-groups 0-2, Pool does group 3
                xn = xn_t[n]
                x8 = xn8_t[n][:, jlo * P:jlo * P + (64 if jlo == NCH - 1
                                                    else 256)].bitcast(F8)
                wtot = 64 if jlo == NCH - 1 else 256
                s0 = jlo * P
                xsq_d = xsqdpool.tile([P, 3, 2 * P], F16, name="xsq_d")
                nc.vector.tensor_tensor(out=xsq_d[:, :, 0:wtot],
                                        in0=xn[:, 0:3, s0:s0 + wtot],
                                        in1=xn[:, 0:3, s0:s0 + wtot], op=OP.mult)
                xsq_p = xsqppool.tile([P, 2 * P], F16, name="xsq_p")
                eng = nc.vector if (dve_only or not USE_GPSIMD) else nc.gpsimd
                eng.tensor_tensor(out=xsq_p[:, 0:wtot], in0=x8, in1=x8,
                                  op=OP.mult)
                xsq_t[(n, jlo)] = (xsq_d, xsq_p, 0)
                if jlo + 1 < NCH:
                    xsq_t[(n, jlo + 1)] = (xsq_d, xsq_p, P)

            def ss_mm(n, j):
                # per-pixel sum-of-squares via ones-matmuls into ss col j
                if sm_t[n] is None:
                    sm_t[n] = ps_sm.tile([P, 32], F32, name="sm")
                ss = sm_t[n][:, 0:16]
                w = 64 if j == NCH - 1 else P
                xsq_d, xsq_p, off = xsq_t.pop((n, j))
                for g in range(3):
                    nc.tensor.matmul(ss[0:w, j:j + 1], xsq_d[:, g, off:off + w],
                                     ones[:, :], start=(g == 0), stop=False)
                nc.tensor.matmul(ss[0:w, j:j + 1], xsq_p[:, off:off + w],
                                 ones[:, :], start=False, stop=True)

            def norm_batch(n, lo, hi):
                if lo == 0:
                    lnss = nrmpool.tile([P, 16], F32, name="lnss")
                    inv_a = nrmpool.tile([P, 16], F32, name="inv_a")
                    nrms = nrmpool.tile([P, 16], F16, name="nrms")
                    sume = nrmpool.tile([P, 16], F32, name="sume")
                    rse = nrmpool.tile([P, 16], F32, name="rse")
                    nrm_t[n] = (lnss, inv_a, nrms, sume, rse)
                lnss, inv_a, nrms, sume, rse = nrm_t[n]
                if hi == NCH:
                    # rows 64:128 of last ss col never written; keep finite
                    nc.vector.memset(sm_t[n][64:P, NCH - 1:NCH], 1.0)
                ss = sm_t[n][:, 0:16]
                nc.scalar.activation(out=lnss[:, lo:hi], in_=ss[:, lo:hi],
                                     func=AF.Ln)
                nc.scalar.activation(out=inv_a[:, lo:hi], in_=lnss[:, lo:hi],
                                     func=AF.Exp, scale=-0.5)
                nc.scalar.activation(out=nrms[:, lo:hi], in_=lnss[:, lo:hi],
                                     func=AF.Exp, scale=0.5)

            load_image(0, fine=True)
            nc.sync.dma_start(wt[:], WT_d[:, :, :])
            nc.sync.dma_start(cent[:], CENT_d[:, :])
            load_image(1, fine=True)
            for p in range(4):
                ss_sq2(0, 2 * p, dve_only=True)
                ss_mm(0, 2 * p)
                ss_mm(0, 2 * p + 1)
            norm_batch(0, 0, 8)
            for p in range(4, 6):
                ss_sq2(0, 2 * p, dve_only=True)
                ss_mm(0, 2 * p)
                ss_mm(0, 2 * p + 1)
            ss_sq2(0, 12, dve_only=True)
            ss_mm(0, 12)
            norm_batch(0, 8, NCH)

            for n in range(NIMG):
                if n + 2 < NIMG:
                    load_image(n + 2)
                xn = xn_t[n]
                xn8 = xn8_t[n]
                xt = xt_t[n]
                _, inv_a, nrms, sume, rse = nrm_t[n]
                agg = ps_agg.tile([K, D], F32, name="agg")
                asum = sm_t[n][0:K, 16:17]

                def emit_agg(pend):
                    e2t, w, j = pend
                    rhs = xt[0:w, j, :]
                    if XT_FMT != "f16":
                        rhs = rhs.bitcast(xt_mm_dt)
                    nc.tensor.matmul(agg[:, :], e2t[0:w, :], rhs,
                                     start=(j == 0), stop=(j == NCH - 1))
                    nc.tensor.matmul(asum[:, :], e2t[0:w, :], nrms[0:w, j:j + 1],
                                     start=(j == 0), stop=(j == NCH - 1))

                pending = []
                expp = None
                for j in range(NCH):
                    # interleave next image's ss work: squares (DVE/Pool) lead
                    # the PE ones-matmuls by 2 chunks so PE never stalls, and
                    # both precede this chunk's exp-dependent chain
                    if n + 1 < NIMG and 6 <= j <= 12:
                        ss_mm(n + 1, j - 6)
                        if j - 6 == 6:
                            ss_mm(n + 1, 7)
                            norm_batch(n + 1, 0, 8)
                    w = 64 if j == NCH - 1 else P
                    s0 = j * P
                    lgp = ps_lg.tile([P, K], F32, name="lgp")
                    for g in range(3):
                        nc.tensor.matmul(lgp[0:w, :], xn[:, g, s0:s0 + w],
                                         wt[:, g, :], start=(g == 0), stop=False)
                    nc.tensor.matmul(lgp[0:w, :],
                                     xn8[:, s0:s0 + w].bitcast(F8),
                                     wt[:, 3, :], start=False, stop=True)
                    expt = epool.tile([P, K], F16, name="expt")
                    if j % 3 == 0:
                        # every 3rd chunk: sumexp via Act's accumulator
                        nc.scalar.activation(out=expt[0:w, :], in_=lgp[0:w, :],
                                             func=AF.Exp,
                                             scale=inv_a[0:w, j:j + 1],
                                             accum_out=sume[0:w, j:j + 1])
                    else:
                        # rest: sumexp via DVE reduce
                        nc.scalar.activation(out=expt[0:w, :], in_=lgp[0:w, :],
                                             func=AF.Exp,
                                             scale=inv_a[0:w, j:j + 1])
                        nc.vector.tensor_reduce(out=sume[0:w, j:j + 1],
                                                in_=expt[0:w, :], axis=AX.X,
                                                op=OP.add)
                    nc.vector.reciprocal(rse[0:w, j:j + 1], sume[0:w, j:j + 1])
                    e2t = e2pool.tile([P, K], F16, name="e2t")
                    nc.vector.tensor_scalar(out=e2t[0:w, :], in0=expt[0:w, :],
                                            scalar1=inv_a[0:w, j:j + 1],
                                            scalar2=rse[0:w, j:j + 1],
                                            op0=OP.mult, op1=OP.mult)
                    pending.append((e2t, w, j))
                    while len(pending) > AGG_DELAY:
                        emit_agg(pending.pop(0))
                    if n + 1 < NIMG:
                        # sq pairs ride at the BOTTOM of the iter: DVE is
                        # in-order, so the exp chain ops must come first
                        if 4 <= j <= 7:
                            ss_sq2(n + 1, 2 * (j - 4))
                        elif 10 <= j <= 11:
                            ss_sq2(n + 1, 2 * (j - 6))
                        elif j == 12:
                            ss_sq2(n + 1, 12)
                for pend in pending:
                    emit_agg(pend)
                if n + 1 < NIMG:
                    for j2 in range(8, NCH):
                        ss_mm(n + 1, j2)
                    norm_batch(n + 1, 8, NCH)

                # ---- finale: nv = asum*cent - agg = -vlad; row-norm; /8 ----
                nv = finpool.tile([K, D], F16, name="nv")
                nvsq = finpool.tile([K, D], F16, name="nvsq")
                sc = finpool.tile([K, 4], F32, name="sc")
                ot = finpool.tile([K, D], F16, name="ot")
                nc.vector.scalar_tensor_tensor(out=nv[:, :], in0=cent[:, :],
                                               scalar=asum[:, 0:1], in1=agg[:, :],
                                               op0=OP.mult, op1=OP.subtract)
                nc.scalar.activation(out=nvsq[:, :], in_=nv[:, :], func=AF.Square,
                                     accum_out=sc[:, 0:1])
                nc.scalar.activation(out=sc[:, 1:2], in_=sc[:, 0:1], func=AF.Ln)
                nc.scalar.activation(out=sc[:, 2:3], in_=sc[:, 1:2], func=AF.Exp,
                                     scale=-0.5, bias=ln8[0:K, 0:1])
                nc.vector.tensor_scalar(out=ot[:, :], in0=nv[:, :],
                                        scalar1=sc[:, 2:3], scalar2=-1.0,
                                        op0=OP.mult, op1=OP.mult)
                nc.sync.dma_start(OUT_d[n, :, :], ot[:, :])
    nc.compile()
    return nc


_NC = None


def _get_nc():
    global _NC
    if _NC is None:
        _NC = build()
    return _NC


def _prep(x, conv_weight, centroids):
    x = np.ascontiguousarray(np.asarray(x), dtype=np.float32)
    w = np.ascontiguousarray(np.asarray(conv_weight), dtype=np.float32)
    c = np.ascontiguousarray(np.asarray(centroids), dtype=np.float32)
    N = x.shape[0]
    # XN: [N, 4g, 128p, S] -> [N, 128p, 4g, S]; groups 0-2 fp16, group 3 fp8
    xn4 = x.reshape(N, 4, P, S).transpose(0, 2, 1, 3)
    xn = np.ascontiguousarray(xn4[:, :, 0:3, :]).astype(np.float16)
    xn8 = np.ascontiguousarray(xn4[:, :, 3, :]).astype(
        ml_dtypes.float8_e3m4).view(np.uint8)
    # XT: pad S to 13*128, [N, D, 13c, 128p] -> [N, 128p, 13c, D] fp8(e3m4)
    xp = np.zeros((N, D, NCH * P), dtype=np.float32)
    xp[:, :, :S] = x.reshape(N, D, S)
    xtr = np.ascontiguousarray(xp.reshape(N, D, NCH, P).transpose(0, 3, 2, 1))
    if XT_FMT == "e3":
        xt8 = xtr.astype(ml_dtypes.float8_e3m4).view(np.uint8)
    elif XT_FMT == "e4":
        xt8 = xtr.astype(ml_dtypes.float8_e4m3).view(np.uint8)
    else:
        xt8 = xtr.astype(np.float16)
    wT3 = np.ascontiguousarray(
        w.reshape(K, 4, P).transpose(2, 1, 0)).astype(np.float16)
    c16 = c.astype(np.float16)
    ones = np.ones((P, 1), dtype=np.float16)
    in_maps = [{"XN": np.ascontiguousarray(xn[NIMG * i:NIMG * (i + 1)]),
                "XN8": np.ascontiguousarray(xn8[NIMG * i:NIMG * (i + 1)]),
                "XT": np.ascontiguousarray(xt8[NIMG * i:NIMG * (i + 1)]),
                "WT": wT3, "CENT": c16, "ONES": ones} for i in range(NCORES)]
    return in_maps


def _run(x, conv_weight, centroids, trace=False):
    from concourse import bass_utils
    nc = _get_nc()
    in_maps = _prep(x, conv_weight, centroids)
    res = bass_utils.run_bass_kernel_spmd(nc, in_maps,
                                          core_ids=list(range(NCORES)),
                                          trace=trace)
    out = np.concatenate(
        [np.asarray(res.results[i]["OUT"]).astype(np.float32).reshape(NIMG, K * D)
         for i in range(NCORES)], axis=0)
    return out, getattr(res, "exec_time_ns", None)


def kernel(x, conv_weight, centroids):
    out, _ = _run(x, conv_weight, centroids, trace=False)
    return out


# revision 99
# speedup vs baseline: 1.0131x; 1.0131x over previous
import sys

for _p in ("/opt/trn_rl_repo", "/opt/trn_rl_repo/concourse"):
    if _p not in sys.path:
        sys.path.insert(0, _p)

import numpy as np
import ml_dtypes
import concourse.bass as bass
import concourse.bacc as bacc
import concourse.mybir as mybir
import concourse.tile as tile

P = 128
D = 512
S = 1600
K = 64
NIMG = 4          # images per core
NCORES = 8
NCH = 13          # 12*128 + 64 = 1600
F32 = mybir.dt.float32
F16 = mybir.dt.float16
F8 = mybir.dt.float8e3   # e3m4
U8 = mybir.dt.uint8
AF = mybir.ActivationFunctionType
OP = mybir.AluOpType
AX = mybir.AxisListType

LN_EIGHTH = -2.0794415416798357  # ln(1/8)
AGG_DELAY = 10
NLG = 8              # logits PSUM slices (one bank)
USE_GPSIMD = True    # offload 4th square group to Pool engine
POOL_UPTO = 13       # Pool handles 4th group only for chunks < POOL_UPTO
XT_FMT = "e3"        # 'e3' | 'e4' | 'f16' for the agg GEMM rhs
SS_OFF = 4           # stagger of next image's ss chunks into current main


def build():
    nc = bacc.Bacc("TRN2", target_bir_lowering=False, debug=False,
                   enable_asserts=True, num_devices=NCORES)
    xt_dram_dt = F16 if XT_FMT == "f16" else U8
    xt_mm_dt = {"e3": F8, "e4": mybir.dt.float8e4, "f16": F16}[XT_FMT]
    # host-relaid layouts:
    #  XN [n, p=d%128, g=d//128, s] fp16   (logits lhsT + squares)
    #  XT [n, p=s%128, c=s//128, d] fp8e3-as-u8  (agg GEMM rhs)
    XN_d = nc.dram_tensor("XN", [NIMG, P, 3, S], F16, kind="ExternalInput").ap()
    XN8_d = nc.dram_tensor("XN8", [NIMG, P, S], U8, kind="ExternalInput").ap()
    XT_d = nc.dram_tensor("XT", [NIMG, P, NCH, D], xt_dram_dt,
                          kind="ExternalInput").ap()
    WT_d = nc.dram_tensor("WT", [P, 4, K], F16, kind="ExternalInput").ap()
    CENT_d = nc.dram_tensor("CENT", [K, D], F16, kind="ExternalInput").ap()
    ONES_d = nc.dram_tensor("ONES", [P, 1], F16, kind="ExternalInput").ap()
    OUT_d = nc.dram_tensor("OUT", [NIMG, K, D], F16, kind="ExternalOutput").ap()

    with tile.TileContext(nc) as tc:
        with tc.tile_pool(name="const", bufs=1) as cpool, \
             tc.tile_pool(name="xn", bufs=3) as xnpool, \
             tc.tile_pool(name="xn8", bufs=3) as xn8pool, \
             tc.tile_pool(name="xt", bufs=3) as xtpool, \
             tc.tile_pool(name="xsqd", bufs=5) as xsqdpool, \
             tc.tile_pool(name="xsqp", bufs=5) as xsqppool, \
             tc.tile_pool(name="nrm", bufs=2) as nrmpool, \
             tc.tile_pool(name="expt", bufs=13) as epool, \
             tc.tile_pool(name="e2", bufs=13) as e2pool, \
             tc.tile_pool(name="fin", bufs=2) as finpool, \
             tc.tile_pool(name="ps_lg", bufs=4, space=bass.MemorySpace.PSUM) as ps_lg, \
             tc.tile_pool(name="ps_agg", bufs=2, space=bass.MemorySpace.PSUM) as ps_agg, \
             tc.tile_pool(name="ps_sm", bufs=2, space=bass.MemorySpace.PSUM) as ps_sm:

            # act set 6 = {ln, exp, square, copy, ...}: one table load total
            nc.scalar.add_instruction(mybir.InstLoadActFuncSet(act_func_set_id=6))
            wt = cpool.tile([P, 4, K], F16)
            cent = cpool.tile([K, D], F16)
            ones = cpool.tile([P, 1], F16)
            ln8 = cpool.tile([P, 1], F32)
            nc.vector.memset(ln8[:], LN_EIGHTH)

            xn_t = [None] * NIMG
            xn8_t = [None] * NIMG
            xt_t = [None] * NIMG
            sm_t = [None] * NIMG
            nrm_t = [None] * NIMG

            def load_image(n, fine=False):
                t = xnpool.tile([P, 3, S], F16, name="xn_t")
                t8 = xn8pool.tile([P, S], U8, name="xn8_t")
                pieces = ((0, 400), (400, 800), (800, 1200), (1200, 1600)) \
                    if fine else ((0, 512), (512, 1024), (1024, 1600))
                for i, (a, b) in enumerate(pieces):
                    nc.sync.dma_start(t[:, :, a:b], XN_d[n, :, :, a:b])
                    if fine and n == 0 and i == 0:
                        # squeeze the tiny ones vector in right after the
                        # first x piece; big consts ride later in the queue
                        nc.sync.dma_start(ones[:], ONES_d[:, :])
                    if i == 0:
                        nc.sync.dma_start(t8[:], XN8_d[n, :, :])
                u = xtpool.tile([P, NCH, D], xt_dram_dt, name="xt_t")
                if fine:
                    nc.sync.dma_start(u[:, 0:7, :], XT_d[n, :, 0:7, :])
                    nc.sync.dma_start(u[:, 7:NCH, :], XT_d[n, :, 7:NCH, :])
                else:
                    nc.sync.dma_start(u[:], XT_d[n, :, :, :])
                xn_t[n] = t
                xn8_t[n] = t8
                xt_t[n] = u

            xsq_t = {}

            def ss_sq2(n, jlo, dve_only=False):
                # squares for a PAIR of chunks [jlo, jlo+1] (last chunk solo):
                # DVE does d-groups 0-2, Pool does group 3
                xn = xn_t[n]
                x8 = xn8_t[n][:, jlo * P:jlo * P + (64 if jlo == NCH - 1
                                                    else 256)].bitcast(F8)
                wtot = 64 if jlo == NCH - 1 else 256
                s0 = jlo * P
                xsq_d = xsqdpool.tile([P, 3, 2 * P], F16, name="xsq_d")
                nc.vector.tensor_tensor(out=xsq_d[:, :, 0:wtot],
                                        in0=xn[:, 0:3, s0:s0 + wtot],
                                        in1=xn[:, 0:3, s0:s0 + wtot], op=OP.mult)
                xsq_p = xsqppool.tile([P, 2 * P], F16, name="xsq_p")
                eng = nc.vector if (dve_only or not USE_GPSIMD) else nc.gpsimd
                eng.tensor_tensor(out=xsq_p[:, 0:wtot], in0=x8, in1=x8,
                                  op=OP.mult)
                xsq_t[(n, jlo)] = (xsq_d, xsq_p, 0)
                if jlo + 1 < NCH:
                    xsq_t[(n, jlo + 1)] = (xsq_d, xsq_p, P)

            def ss_mm(n, j):
                # per-pixel sum-of-squares via ones-matmuls into ss col j
                if sm_t[n] is None:
                    sm_t[n] = ps_sm.tile([P, 32], F32, name="sm")
                ss = sm_t[n][:, 0:16]
                w = 64 if j == NCH - 1 else P
                xsq_d, xsq_p, off = xsq_t.pop((n, j))
                for g in range(3):
                    nc.tensor.matmul(ss[0:w, j:j + 1], xsq_d[:, g, off:off + w],
                                     ones[:, :], start=(g == 0), stop=False)
                nc.tensor.matmul(ss[0:w, j:j + 1], xsq_p[:, off:off + w],
                                 ones[:, :], start=False, stop=True)

            def norm_batch(n, lo, hi):
                if lo == 0:
                    lnss = nrmpool.tile([P, 16], F32, name="lnss")
                    inv_a = nrmpool.tile([P, 16], F32, name="inv_a")
                    nrms = nrmpool.tile([P, 16], F16, name="nrms")
                    sume = nrmpool.tile([P, 16], F32, name="sume")
                    rse = nrmpool.tile([P, 16], F32, name="rse")
                    nrm_t[n] = (lnss, inv_a, nrms, sume, rse)
                lnss, inv_a, nrms, sume, rse = nrm_t[n]
                if hi == NCH:
                    # rows 64:128 of last ss col never written; keep finite
                    nc.vector.memset(sm_t[n][64:P, NCH - 1:NCH], 1.0)
                ss = sm_t[n][:, 0:16]
                nc.scalar.activation(out=lnss[:, lo:hi], in_=ss[:, lo:hi],
                                     func=AF.Ln)
                nc.scalar.activation(out=inv_a[:, lo:hi], in_=lnss[:, lo:hi],
                                     func=AF.Exp, scale=-0.5)
                nc.scalar.activation(out=nrms[:, lo:hi], in_=lnss[:, lo:hi],
                                     func=AF.Exp, scale=0.5)

            load_image(0, fine=True)
            nc.sync.dma_start(wt[:], WT_d[:, :, :])
            nc.sync.dma_start(cent[:], CENT_d[:, :])
            load_image(1, fine=True)
            for p in range(4):
                ss_sq2(0, 2 * p, dve_only=True)
                ss_mm(0, 2 * p)
                ss_mm(0, 2 * p + 1)
            norm_batch(0, 0, 8)
            for p in range(4, 6):
                ss_sq2(0, 2 * p, dve_only=True)
                ss_mm(0, 2 * p)
                ss_mm(0, 2 * p + 1)
            ss_sq2(0, 12, dve_only=True)
            ss_mm(0, 12)
            norm_batch(0, 8, NCH)

            for n in range(NIMG):
                if n + 2 < NIMG:
                    load_image(n + 2)
                xn = xn_t[n]
                xn8 = xn8_t[n]
                xt = xt_t[n]
                _, inv_a, nrms, sume, rse = nrm_t[n]
                agg = ps_agg.tile([K, D], F32, name="agg")
                asum = sm_t[n][0:K, 16:17]

                def emit_agg(pend):
                    e2t, w, j = pend
                    rhs = xt[0:w, j, :]
                    if XT_FMT != "f16":
                        rhs = rhs.bitcast(xt_mm_dt)
                    nc.tensor.matmul(agg[:, :], e2t[0:w, :], rhs,
                                     start=(j == 0), stop=(j == NCH - 1))
                    nc.tensor.matmul(asum[:, :], e2t[0:w, :], nrms[0:w, j:j + 1],
                                     start=(j == 0), stop=(j == NCH - 1))

                pending = []
                expp = None
                for j in range(NCH):
                    # interleave next image's ss work: squares (DVE/Pool) lead
                    # the PE ones-matmuls by 2 chunks so PE never stalls, and
                    # both precede this chunk's exp-dependent chain
                    if n + 1 < NIMG and 6 <= j <= 12:
                        ss_mm(n + 1, j - 6)
                        if j - 6 == 6:
                            ss_mm(n + 1, 7)
                            norm_batch(n + 1, 0, 8)
                    w = 64 if j == NCH - 1 else P
                    s0 = j * P
                    lgp = ps_lg.tile([P, K], F32, name="lgp")
                    for g in range(3):
                        nc.tensor.matmul(lgp[0:w, :], xn[:, g, s0:s0 + w],
                                         wt[:, g, :], start=(g == 0), stop=False)
                    nc.tensor.matmul(lgp[0:w, :],
                                     xn8[:, s0:s0 + w].bitcast(F8),
                                     wt[:, 3, :], start=False, stop=True)
                    expt = epool.tile([P, K], F16, name="expt")
                    if j % 3 == 0 and n < NIMG - 1:
                        # every 3rd chunk: sumexp via Act's accumulator
                        nc.scalar.activation(out=expt[0:w, :], in_=lgp[0:w, :],
                                             func=AF.Exp,
                                             scale=inv_a[0:w, j:j + 1],
                                             accum_out=sume[0:w, j:j + 1])
                    else:
                        # rest: sumexp via DVE reduce
                        nc.scalar.activation(out=expt[0:w, :], in_=lgp[0:w, :],
                                             func=AF.Exp,
                                             scale=inv_a[0:w, j:j + 1])
                        nc.vector.tensor_reduce(out=sume[0:w, j:j + 1],
                                                in_=expt[0:w, :], axis=AX.X,
                                                op=OP.add)
                    nc.vector.reciprocal(rse[0:w, j:j + 1], sume[0:w, j:j + 1])
                    e2t = e2pool.tile([P, K], F16, name="e2t")
                    nc.vector.tensor_scalar(out=e2t[0:w, :], in0=expt[0:w, :],
                                            scalar1=inv_a[0:w, j:j + 1],
                                            scalar2=rse[0:w, j:j + 1],
                                            op0=OP.mult, op1=OP.mult)
                    pending.append((e2t, w, j))
                    while len(pending) > AGG_DELAY:
                        emit_agg(pending.pop(0))
                    if n + 1 < NIMG:
                        # sq pairs ride at the BOTTOM of the iter: DVE is
                        # in-order, so the exp chain ops must come first
                        if 4 <= j <= 7:
                            ss_sq2(n + 1, 2 * (j - 4))
                        elif 10 <= j <= 11:
                            ss_sq2(n + 1, 2 * (j - 6))
                        elif j == 12:
                            ss_sq2(n + 1, 12)
                for pend in pending:
                    emit_agg(pend)
                if n + 1 < NIMG:
                    for j2 in range(8, NCH):
                        ss_mm(n + 1, j2)
                    norm_batch(n + 1, 8, NCH)

                # ---- finale: nv = asum*cent - agg = -vlad; row-norm; /8 ----
                nv = finpool.tile([K, D], F16, name="nv")
                nvsq = finpool.tile([K, D], F16, name="nvsq")
                sc = finpool.tile([K, 4], F32, name="sc")
                ot = finpool.tile([K, D], F16, name="ot")
                nc.vector.scalar_tensor_tensor(out=nv[:, :], in0=cent[:, :],
                                               scalar=asum[:, 0:1], in1=agg[:, :],
                                               op0=OP.mult, op1=OP.subtract)
                nc.scalar.activation(out=nvsq[:, :], in_=nv[:, :], func=AF.Square,
                                     accum_out=sc[:, 0:1])
                nc.scalar.activation(out=sc[:, 1:2], in_=sc[:, 0:1], func=AF.Ln)
                nc.scalar.activation(out=sc[:, 2:3], in_=sc[:, 1:2], func=AF.Exp,
                                     scale=-0.5, bias=ln8[0:K, 0:1])
                nc.vector.tensor_scalar(out=ot[:, :], in0=nv[:, :],
                                        scalar1=sc[:, 2:3], scalar2=-1.0,
                                        op0=OP.mult, op1=OP.mult)
                nc.sync.dma_start(OUT_d[n, :, :], ot[:, :])
    nc.compile()
    return nc


_NC = None


def _get_nc():
    global _NC
    if _NC is None:
        _NC = build()
    return _NC


def _prep(x, conv_weight, centroids):
    x = np.ascontiguousarray(np.asarray(x), dtype=np.float32)
    w = np.ascontiguousarray(np.asarray(conv_weight), dtype=np.float32)
    c = np.ascontiguousarray(np.asarray(centroids), dtype=np.float32)
    N = x.shape[0]
    # XN: [N, 4g, 128p, S] -> [N, 128p, 4g, S]; groups 0-2 fp16, group 3 fp8
    xn4 = x.reshape(N, 4, P, S).transpose(0, 2, 1, 3)
    xn = np.ascontiguousarray(xn4[:, :, 0:3, :]).astype(np.float16)
    xn8 = np.ascontiguousarray(xn4[:, :, 3, :]).astype(
        ml_dtypes.float8_e3m4).view(np.uint8)
    # XT: pad S to 13*128, [N, D, 13c, 128p] -> [N, 128p, 13c, D] fp8(e3m4)
    xp = np.zeros((N, D, NCH * P), dtype=np.float32)
    xp[:, :, :S] = x.reshape(N, D, S)
    xtr = np.ascontiguousarray(xp.reshape(N, D, NCH, P).transpose(0, 3, 2, 1))
    if XT_FMT == "e3":
        xt8 = xtr.astype(ml_dtypes.float8_e3m4).view(np.uint8)
    elif XT_FMT == "e4":
        xt8 = xtr.astype(ml_dtypes.float8_e4m3).view(np.uint8)
    else:
        xt8 = xtr.astype(np.float16)
    wT3 = np.ascontiguousarray(
        w.reshape(K, 4, P).transpose(2, 1, 0)).astype(np.float16)
    c16 = c.astype(np.float16)
    ones = np.ones((P, 1), dtype=np.float16)
    in_maps = [{"XN": np.ascontiguousarray(xn[NIMG * i:NIMG * (i + 1)]),
                "XN8": np.ascontiguousarray(xn8[NIMG * i:NIMG * (i + 1)]),
                "XT": np.ascontiguousarray(xt8[NIMG * i:NIMG * (i + 1)]),
                "WT": wT3, "CENT": c16, "ONES": ones} for i in range(NCORES)]
    return in_maps


def _run(x, conv_weight, centroids, trace=False):
    from concourse import bass_utils
    nc = _get_nc()
    in_maps = _prep(x, conv_weight, centroids)
    res = bass_utils.run_bass_kernel_spmd(nc, in_maps,
                                          core_ids=list(range(NCORES)),
                                          trace=trace)
    out = np.concatenate(
        [np.asarray(res.results[i]["OUT"]).astype(np.float32).reshape(NIMG, K * D)
         for i in range(NCORES)], axis=0)
    return out, getattr(res, "exec_time_ns", None)


def kernel(x, conv_weight, centroids):
    out, _ = _run(x, conv_weight, centroids, trace=False)
    return out


# revision 103
# speedup vs baseline: 1.0174x; 1.0042x over previous
import sys

for _p in ("/opt/trn_rl_repo", "/opt/trn_rl_repo/concourse"):
    if _p not in sys.path:
        sys.path.insert(0, _p)

import numpy as np
import ml_dtypes
import concourse.bass as bass
import concourse.bacc as bacc
import concourse.mybir as mybir
import concourse.tile as tile

P = 128
D = 512
S = 1600
K = 64
NIMG = 4          # images per core
NCORES = 8
NCH = 13          # 12*128 + 64 = 1600
F32 = mybir.dt.float32
F16 = mybir.dt.float16
F8 = mybir.dt.float8e3   # e3m4
U8 = mybir.dt.uint8
AF = mybir.ActivationFunctionType
OP = mybir.AluOpType
AX = mybir.AxisListType

LN_EIGHTH = -2.0794415416798357  # ln(1/8)
AGG_DELAY = 10
NLG = 8              # logits PSUM slices (one bank)
USE_GPSIMD = True    # offload 4th square group to Pool engine
POOL_UPTO = 13       # Pool handles 4th group only for chunks < POOL_UPTO
XT_FMT = "e3"        # 'e3' | 'e4' | 'f16' for the agg GEMM rhs
SS_OFF = 4           # stagger of next image's ss chunks into current main


def build():
    nc = bacc.Bacc("TRN2", target_bir_lowering=False, debug=False,
                   enable_asserts=True, num_devices=NCORES)
    xt_dram_dt = F16 if XT_FMT == "f16" else U8
    xt_mm_dt = {"e3": F8, "e4": mybir.dt.float8e4, "f16": F16}[XT_FMT]
    # host-relaid layouts:
    #  XN [n, p=d%128, g=d//128, s] fp16   (logits lhsT + squares)
    #  XT [n, p=s%128, c=s//128, d] fp8e3-as-u8  (agg GEMM rhs)
    XN_d = nc.dram_tensor("XN", [NIMG, P, 3, S], F16, kind="ExternalInput").ap()
    XN8_d = nc.dram_tensor("XN8", [NIMG, P, S], U8, kind="ExternalInput").ap()
    XT_d = nc.dram_tensor("XT", [NIMG, P, NCH, D], xt_dram_dt,
                          kind="ExternalInput").ap()
    WT_d = nc.dram_tensor("WT", [P, 4, K], F16, kind="ExternalInput").ap()
    CENT_d = nc.dram_tensor("CENT", [K, D], F16, kind="ExternalInput").ap()
    ONES_d = nc.dram_tensor("ONES", [P, 1], F16, kind="ExternalInput").ap()
    OUT_d = nc.dram_tensor("OUT", [NIMG, K, D], F16, kind="ExternalOutput").ap()

    with tile.TileContext(nc) as tc:
        with tc.tile_pool(name="const", bufs=1) as cpool, \
             tc.tile_pool(name="xn", bufs=3) as xnpool, \
             tc.tile_pool(name="xn8", bufs=3) as xn8pool, \
             tc.tile_pool(name="xt", bufs=3) as xtpool, \
             tc.tile_pool(name="xsqd", bufs=7) as xsqdpool, \
             tc.tile_pool(name="xsqp", bufs=7) as xsqppool, \
             tc.tile_pool(name="nrm", bufs=2) as nrmpool, \
             tc.tile_pool(name="expt", bufs=13) as epool, \
             tc.tile_pool(name="e2", bufs=13) as e2pool, \
             tc.tile_pool(name="fin", bufs=2) as finpool, \
             tc.tile_pool(name="ps_lg", bufs=4, space=bass.MemorySpace.PSUM) as ps_lg, \
             tc.tile_pool(name="ps_agg", bufs=2, space=bass.MemorySpace.PSUM) as ps_agg, \
             tc.tile_pool(name="ps_sm", bufs=2, space=bass.MemorySpace.PSUM) as ps_sm:

            # act set 6 = {ln, exp, square, copy, ...}: one table load total
            nc.scalar.add_instruction(mybir.InstLoadActFuncSet(act_func_set_id=6))
            wt = cpool.tile([P, 4, K], F16)
            cent = cpool.tile([K, D], F16)
            ones = cpool.tile([P, 1], F16)
            ln8 = cpool.tile([P, 1], F32)
            nc.vector.memset(ln8[:], LN_EIGHTH)

            xn_t = [None] * NIMG
            xn8_t = [None] * NIMG
            xt_t = [None] * NIMG
            sm_t = [None] * NIMG
            nrm_t = [None] * NIMG

            def load_image(n, fine=False):
                t = xnpool.tile([P, 3, S], F16, name="xn_t")
                t8 = xn8pool.tile([P, S], U8, name="xn8_t")
                pieces = ((0, 400), (400, 800), (800, 1200), (1200, 1600)) \
                    if fine else ((0, 512), (512, 1024), (1024, 1600))
                for i, (a, b) in enumerate(pieces):
                    nc.sync.dma_start(t[:, :, a:b], XN_d[n, :, :, a:b])
                    if fine and n == 0 and i == 0:
                        # squeeze the tiny ones vector in right after the
                        # first x piece; big consts ride later in the queue
                        nc.sync.dma_start(ones[:], ONES_d[:, :])
                    if i == 0:
                        nc.sync.dma_start(t8[:], XN8_d[n, :, :])
                u = xtpool.tile([P, NCH, D], xt_dram_dt, name="xt_t")
                if fine:
                    nc.sync.dma_start(u[:, 0:7, :], XT_d[n, :, 0:7, :])
                    nc.sync.dma_start(u[:, 7:NCH, :], XT_d[n, :, 7:NCH, :])
                else:
                    nc.sync.dma_start(u[:], XT_d[n, :, :, :])
                xn_t[n] = t
                xn8_t[n] = t8
                xt_t[n] = u

            xsq_t = {}

            def ss_sq2(n, jlo, dve_only=False):
                # squares for a PAIR of chunks [jlo, jlo+1] (last chunk solo):
                # DVE does d-groups 0-2, Pool does group 3
                xn = xn_t[n]
                x8 = xn8_t[n][:, jlo * P:jlo * P + (64 if jlo == NCH - 1
                                                    else 256)].bitcast(F8)
                wtot = 64 if jlo == NCH - 1 else 256
                s0 = jlo * P
                xsq_d = xsqdpool.tile([P, 3, 2 * P], F16, name="xsq_d")
                nc.vector.tensor_tensor(out=xsq_d[:, :, 0:wtot],
                                        in0=xn[:, 0:3, s0:s0 + wtot],
                                        in1=xn[:, 0:3, s0:s0 + wtot], op=OP.mult)
                xsq_p = xsqppool.tile([P, 2 * P], F16, name="xsq_p")
                if dve_only:
                    nc.scalar.activation(out=xsq_p[:, 0:wtot], in_=x8,
                                         func=AF.Square)
                else:
                    eng = nc.gpsimd if USE_GPSIMD else nc.vector
                    eng.tensor_tensor(out=xsq_p[:, 0:wtot], in0=x8, in1=x8,
                                      op=OP.mult)
                xsq_t[(n, jlo)] = (xsq_d, xsq_p, 0)
                if jlo + 1 < NCH:
                    xsq_t[(n, jlo + 1)] = (xsq_d, xsq_p, P)

            def ss_mm(n, j):
                # per-pixel sum-of-squares via ones-matmuls into ss col j
                if sm_t[n] is None:
                    sm_t[n] = ps_sm.tile([P, 32], F32, name="sm")
                ss = sm_t[n][:, 0:16]
                w = 64 if j == NCH - 1 else P
                xsq_d, xsq_p, off = xsq_t.pop((n, j))
                for g in range(3):
                    nc.tensor.matmul(ss[0:w, j:j + 1], xsq_d[:, g, off:off + w],
                                     ones[:, :], start=(g == 0), stop=False)
                nc.tensor.matmul(ss[0:w, j:j + 1], xsq_p[:, off:off + w],
                                 ones[:, :], start=False, stop=True)

            def norm_batch(n, lo, hi):
                if lo == 0:
                    lnss = nrmpool.tile([P, 16], F32, name="lnss")
                    inv_a = nrmpool.tile([P, 16], F32, name="inv_a")
                    nrms = nrmpool.tile([P, 16], F16, name="nrms")
                    sume = nrmpool.tile([P, 16], F32, name="sume")
                    rse = nrmpool.tile([P, 16], F32, name="rse")
                    nrm_t[n] = (lnss, inv_a, nrms, sume, rse)
                lnss, inv_a, nrms, sume, rse = nrm_t[n]
                if hi == NCH:
                    # rows 64:128 of last ss col never written; keep finite
                    nc.vector.memset(sm_t[n][64:P, NCH - 1:NCH], 1.0)
                ss = sm_t[n][:, 0:16]
                nc.scalar.activation(out=lnss[:, lo:hi], in_=ss[:, lo:hi],
                                     func=AF.Ln)
                nc.scalar.activation(out=inv_a[:, lo:hi], in_=lnss[:, lo:hi],
                                     func=AF.Exp, scale=-0.5)
                nc.scalar.activation(out=nrms[:, lo:hi], in_=lnss[:, lo:hi],
                                     func=AF.Exp, scale=0.5)

            load_image(0, fine=True)
            nc.sync.dma_start(wt[:], WT_d[:, :, :])
            nc.sync.dma_start(cent[:], CENT_d[:, :])
            load_image(1, fine=True)
            for p in range(4):
                ss_sq2(0, 2 * p)
                ss_mm(0, 2 * p)
                ss_mm(0, 2 * p + 1)
            norm_batch(0, 0, 8)
            for p in range(4, 6):
                ss_sq2(0, 2 * p)
                ss_mm(0, 2 * p)
                ss_mm(0, 2 * p + 1)
            ss_sq2(0, 12)
            ss_mm(0, 12)
            norm_batch(0, 8, NCH)

            for n in range(NIMG):
                if n + 2 < NIMG:
                    load_image(n + 2)
                xn = xn_t[n]
                xn8 = xn8_t[n]
                xt = xt_t[n]
                _, inv_a, nrms, sume, rse = nrm_t[n]
                agg = ps_agg.tile([K, D], F32, name="agg")
                asum = sm_t[n][0:K, 16:17]

                def emit_agg(pend):
                    e2t, w, j = pend
                    rhs = xt[0:w, j, :]
                    if XT_FMT != "f16":
                        rhs = rhs.bitcast(xt_mm_dt)
                    nc.tensor.matmul(agg[:, :], e2t[0:w, :], rhs,
                                     start=(j == 0), stop=(j == NCH - 1))
                    nc.tensor.matmul(asum[:, :], e2t[0:w, :], nrms[0:w, j:j + 1],
                                     start=(j == 0), stop=(j == NCH - 1))

                pending = []
                expp = None
                for j in range(NCH):
                    # interleave next image's ss work: squares (DVE/Pool) lead
                    # the PE ones-matmuls by 2 chunks so PE never stalls, and
                    # both precede this chunk's exp-dependent chain
                    if n + 1 < NIMG and 6 <= j <= 12:
                        ss_mm(n + 1, j - 6)
                        if j - 6 == 6:
                            ss_mm(n + 1, 7)
                            norm_batch(n + 1, 0, 8)
                    w = 64 if j == NCH - 1 else P
                    s0 = j * P
                    lgp = ps_lg.tile([P, K], F32, name="lgp")
                    for g in range(3):
                        nc.tensor.matmul(lgp[0:w, :], xn[:, g, s0:s0 + w],
                                         wt[:, g, :], start=(g == 0), stop=False)
                    nc.tensor.matmul(lgp[0:w, :],
                                     xn8[:, s0:s0 + w].bitcast(F8),
                                     wt[:, 3, :], start=False, stop=True)
                    expt = epool.tile([P, K], F16, name="expt")
                    if j % 3 == 0 and n < NIMG - 1:
                        # every 3rd chunk: sumexp via Act's accumulator
                        nc.scalar.activation(out=expt[0:w, :], in_=lgp[0:w, :],
                                             func=AF.Exp,
                                             scale=inv_a[0:w, j:j + 1],
                                             accum_out=sume[0:w, j:j + 1])
                    else:
                        # rest: sumexp via DVE reduce
                        nc.scalar.activation(out=expt[0:w, :], in_=lgp[0:w, :],
                                             func=AF.Exp,
                                             scale=inv_a[0:w, j:j + 1])
                        nc.vector.tensor_reduce(out=sume[0:w, j:j + 1],
                                                in_=expt[0:w, :], axis=AX.X,
                                                op=OP.add)
                    nc.vector.reciprocal(rse[0:w, j:j + 1], sume[0:w, j:j + 1])
                    e2t = e2pool.tile([P, K], F16, name="e2t")
                    nc.vector.tensor_scalar(out=e2t[0:w, :], in0=expt[0:w, :],
                                            scalar1=inv_a[0:w, j:j + 1],
                                            scalar2=rse[0:w, j:j + 1],
                                            op0=OP.mult, op1=OP.mult)
                    pending.append((e2t, w, j))
                    while len(pending) > AGG_DELAY:
                        emit_agg(pending.pop(0))
                    if n + 1 < NIMG:
                        # sq pairs ride at the BOTTOM of the iter: DVE is
                        # in-order, so the exp chain ops must come first
                        if 4 <= j <= 7:
                            ss_sq2(n + 1, 2 * (j - 4))
                        elif 10 <= j <= 11:
                            ss_sq2(n + 1, 2 * (j - 6))
                        elif j == 12:
                            ss_sq2(n + 1, 12)
                for pend in pending:
                    emit_agg(pend)
                if n + 1 < NIMG:
                    for j2 in range(8, NCH):
                        ss_mm(n + 1, j2)
                    norm_batch(n + 1, 8, NCH)

                # ---- finale: nv = asum*cent - agg = -vlad; row-norm; /8 ----
                nv = finpool.tile([K, D], F16, name="nv")
                nvsq = finpool.tile([K, D], F16, name="nvsq")
                sc = finpool.tile([K, 4], F32, name="sc")
                ot = finpool.tile([K, D], F16, name="ot")
                nc.vector.scalar_tensor_tensor(out=nv[:, :], in0=cent[:, :],
                                               scalar=asum[:, 0:1], in1=agg[:, :],
                                               op0=OP.mult, op1=OP.subtract)
                nc.scalar.activation(out=nvsq[:, :], in_=nv[:, :], func=AF.Square,
                                     accum_out=sc[:, 0:1])
                nc.scalar.activation(out=sc[:, 1:2], in_=sc[:, 0:1], func=AF.Ln)
                nc.scalar.activation(out=sc[:, 2:3], in_=sc[:, 1:2], func=AF.Exp,
                                     scale=-0.5, bias=ln8[0:K, 0:1])
                nc.vector.tensor_scalar(out=ot[:, :], in0=nv[:, :],
                                        scalar1=sc[:, 2:3], scalar2=-1.0,
                                        op0=OP.mult, op1=OP.mult)
                nc.sync.dma_start(OUT_d[n, :, :], ot[:, :])
    nc.compile()
    return nc


_NC = None


def _get_nc():
    global _NC
    if _NC is None:
        _NC = build()
    return _NC


def _prep(x, conv_weight, centroids):
    x = np.ascontiguousarray(np.asarray(x), dtype=np.float32)
    w = np.ascontiguousarray(np.asarray(conv_weight), dtype=np.float32)
    c = np.ascontiguousarray(np.asarray(centroids), dtype=np.float32)
    N = x.shape[0]
    # XN: [N, 4g, 128p, S] -> [N, 128p, 4g, S]; groups 0-2 fp16, group 3 fp8
    xn4 = x.reshape(N, 4, P, S).transpose(0, 2, 1, 3)
    xn = np.ascontiguousarray(xn4[:, :, 0:3, :]).astype(np.float16)
    xn8 = np.ascontiguousarray(xn4[:, :, 3, :]).astype(
        ml_dtypes.float8_e3m4).view(np.uint8)
    # XT: pad S to 13*128, [N, D, 13c, 128p] -> [N, 128p, 13c, D] fp8(e3m4)
    xp = np.zeros((N, D, NCH * P), dtype=np.float32)
    xp[:, :, :S] = x.reshape(N, D, S)
    xtr = np.ascontiguousarray(xp.reshape(N, D, NCH, P).transpose(0, 3, 2, 1))
    if XT_FMT == "e3":
        xt8 = xtr.astype(ml_dtypes.float8_e3m4).view(np.uint8)
    elif XT_FMT == "e4":
        xt8 = xtr.astype(ml_dtypes.float8_e4m3).view(np.uint8)
    else:
        xt8 = xtr.astype(np.float16)
    wT3 = np.ascontiguousarray(
        w.reshape(K, 4, P).transpose(2, 1, 0)).astype(np.float16)
    c16 = c.astype(np.float16)
    ones = np.ones((P, 1), dtype=np.float16)
    in_maps = [{"XN": np.ascontiguousarray(xn[NIMG * i:NIMG * (i + 1)]),
                "XN8": np.ascontiguousarray(xn8[NIMG * i:NIMG * (i + 1)]),
                "XT": np.ascontiguousarray(xt8[NIMG * i:NIMG * (i + 1)]),
                "WT": wT3, "CENT": c16, "ONES": ones} for i in range(NCORES)]
    return in_maps


def _run(x, conv_weight, centroids, trace=False):
    from concourse import bass_utils
    nc = _get_nc()
    in_maps = _prep(x, conv_weight, centroids)
    res = bass_utils.run_bass_kernel_spmd(nc, in_maps,
                                          core_ids=list(range(NCORES)),
                                          trace=trace)
    out = np.concatenate(
        [np.asarray(res.results[i]["OUT"]).astype(np.float32).reshape(NIMG, K * D)
         for i in range(NCORES)], axis=0)
    return out, getattr(res, "exec_time_ns", None)


def kernel(x, conv_weight, centroids):
    out, _ = _run(x, conv_weight, centroids, trace=False)
    return out
